# revision 37
# baseline (speedup 1.0000x reference)
"""Trainium2 Bass kernel for nn_CollectiveDecActorTaxi0Obs (gnn_message_passing).

Computes, for obs [32768, 48], per-zone dense heads W [81, 48, 5] (+bias b,
adjacency idx/mask [81, 5]):
    logits = einsum('bd,ndk->bnk', obs, W) + b ; masked softmax over k
    out[b, n, idx[n, k]] += probs[b, n, k]              -> [32768, 81, 81] f32

Strategy (pure data parallelism, 8 cores, batch-sharded 4096 rows each):
  The kernel is HBM-write-bound: the output is 860 MB dense but within the
  2e-2 tolerance, so the device writes it as u8 fixed-point (prob*254,
  max quant err 0.5/254 ~ 0.002 << 0.019 tolerance; DVE f32->u8 conversion
  rounds-to-nearest-even and saturates). 215 MB total, ~27 MB/core, ~75 us
  at the ~358 GB/s per-core HBM limit. The host dequantizes via LUT.

  Everything runs with batch on the PARTITION dim in 32 sub-blocks of 128
  rows per core:
    - logits: one [49,128]^T @ [49,405] f32 matmul per sub-block (weights
      Wa pack all 81 zones' 5 slot columns + a bias row; masked slots get
      bias -1e9 so exp underflows to exactly 0).
    - exp on the scalar engine (PSUM -> SBUF), per-zone denominator via a
      window-5 tensor_reduce on GPSIMD + scale by 1/254 there, reciprocal
      on DVE (so rc = 254/den).
    - The scatter out[b, n, idx[n,k]] is batch-invariant: only ~405 of the
      6561 output columns are ever nonzero. Output tiles [128, 6561] u8
      live persistently in SBUF, memset to zero ONCE (halves split across
      DVE/GPSIMD, interleaved with the first sub-blocks); each sub-block
      just rewrites the hot columns with strided e*rc ops (dst stride 82
      on the zone-diagonal, classes hull-extended to single runs by writing
      computed zeros over never-hot columns), all on DVE (GPSIMD cannot
      convert f32->u8: integer TT on Pool requires matching dtypes), then
      DMAs the dense tile. For the grid adjacency this is 5 flat strided
      ops per sub-block.

  The host plans slot classes generically from idx/mask; if a zone has
  duplicate destinations (scatter-add semantics), it falls back to a dense
  scatter-matmul path (probs @ 0/1 S matrix, f32 output).
"""

import os
import sys

sys.path.insert(0, "/opt/trn_rl_repo")

import numpy as np

NZ = 81          # zones
D = 48           # obs dim used
DA = D + 1       # + bias row
KADJ = 5         # adjacency slots per zone
NCORES = 8
BATCH = 32768
BLOC = BATCH // NCORES   # 4096 rows per core
P = 128
NSUB = BLOC // P         # 32 sub-blocks of 128 batch rows
SLOTS = NZ * KADJ        # 405 packed slot columns
SLOTSP = 512             # padded slot pitch: one PSUM bank of f32 per half
OUTW = NZ * NZ           # 6561 output columns
PADL = 4                 # osb left pad: lets merged ops write col -1..-4
PADR = 3                 # osb right pad (also rounds width to mult of 4)
OSBW = PADL + OUTW + PADR
NOSB = 6                 # persistent output staging buffers (block pairs)
NEG = np.float32(-1e9)
QSCALE = np.float32(254.0)  # u8 fixed-point scale for probs in [0, 1]

LAST_RESULTS = None


# --------------------------------------------------------------------------
# Fast path: class-slot planning + strided-scatter program
# --------------------------------------------------------------------------

def _plan_scatter(idx, mask):
    """Assign each valid (zone, k) a slot class c so that zones sharing a
    destination offset o = idx-n share c, then group (o, c) classes into
    strided ops. Returns (assign, ops) or None if any zone has duplicate
    destinations (needs scatter-ADD, handled by the fallback path).

    assign: {n: {c: k}}   ops: [{o, c, z0, L, R, s}] meaning zones
    z0 + i*s + j for i<R, j<L write probs[:, 5*(z)+c] to out col 82*z + o.
    """
    from collections import Counter

    byzone = {}
    for n in range(NZ):
        dests = set()
        for k in range(KADJ):
            if mask[n, k] > 0:
                d = int(idx[n, k])
                if d in dests:
                    return None
                dests.add(d)
                byzone.setdefault(n, []).append((k, d - n))

    # Slot assignment: the (up to 5) globally most common offsets get slot
    # index = their rank in ASCENDING offset order, so classes with adjacent
    # offsets sit in adjacent slots and can later chain into one op.
    cnt = Counter(o for lst in byzone.values() for (_, o) in lst)
    top = [o for o, _ in cnt.most_common(KADJ)]
    pref = {o: r for r, o in enumerate(sorted(top))}

    assign = {n: {} for n in range(NZ)}
    offs = {n: set(o for (_, o) in byzone.get(n, [])) for n in range(NZ)}
    classes = {}
    for n in range(NZ):
        used, rest = set(), []
        for k, o in byzone.get(n, []):
            c = pref.get(o, KADJ)
            if c < KADJ and c not in used:
                used.add(c)
                assign[n][c] = k
                classes.setdefault((o, c), []).append(n)
            else:
                rest.append((k, o))
        free = [c for c in range(KADJ) if c not in used]
        for (k, o), c in zip(rest, free):
            assign[n][c] = k
            classes.setdefault((o, c), []).append(n)

    def cell_ok(o, c, z, zone_set):
        """May an op write cell (z, 82z+o+PADL) from slot (z, 5z+c)? Yes if
        z is a class member; else we'd write a computed zero (slot c must be
        unassigned there so Wa bias -1e9 -> exp 0), the zone must have some
        valid slot (else rc is inf -> 0*inf = NaN), and an in-row cell must
        not shadow another slot's destination column. Out-of-row cells land
        in the osb pad bytes (never DMA'd) and are always harmless."""
        col = (NZ + 1) * z + o
        if col < -PADL or col > OUTW - 1 + PADR:
            return False
        if z in zone_set:
            return True
        if not byzone.get(z):
            return False
        if c in assign[z]:
            return False
        if col < 0 or col > OUTW - 1:
            return True
        return o not in offs[z]

    # Per-instruction fixed cost (~0.4-0.6 us) dominates these ops, so fold
    # as many classes as possible into single instructions:
    #  Pass 1  C-chain merge: classes (o0+j, c0+j) share one op whose inner
    #          dim steps both src slot and dst column by 1 (contiguous).
    #  Pass 2  leftover classes -> single-run hull or uniform runs.
    #  Pass 3  R-merge ops with equal (L, C) via independent per-AP strides.
    merged_ops = []
    consumed = set()
    items = sorted(classes.items())
    cls = {oc: sorted(zs) for oc, zs in items}
    keys = set(cls)
    for (o0, c0) in sorted(keys):
        if (o0, c0) in consumed:
            continue
        chain = [(o0, c0)]
        while (chain[-1][0] + 1, chain[-1][1] + 1) in keys and (
            chain[-1][0] + 1, chain[-1][1] + 1
        ) not in consumed:
            chain.append((chain[-1][0] + 1, chain[-1][1] + 1))
        while len(chain) >= 2:
            A = min(cls[oc][0] for oc in chain)
            B = max(cls[oc][-1] for oc in chain)
            if all(
                cell_ok(o, c, z, set(cls[(o, c)]))
                for (o, c) in chain
                for z in range(A, B + 1)
            ):
                merged_ops.append(
                    dict(o=o0, c=c0, z0=A, L=B - A + 1, R=1, C=len(chain),
                         dstep=1, rs_src=0, rs_dst=0, rs_rc=0)
                )
                consumed.update(chain)
                break
            chain.pop()

    flat = []
    for (o, c), zones in items:
        if (o, c) in consumed:
            continue
        zone_set = set(zones)
        a, b = zones[0], zones[-1]
        if all(cell_ok(o, c, z, zone_set) for z in range(a, b + 1)):
            flat.append(dict(o=o, c=c, z0=a, L=b - a + 1, R=1, C=1,
                             dstep=0, rs_src=0, rs_dst=0, rs_rc=0))
            continue
        runs, z0, prev = [], zones[0], zones[0]
        for z in zones[1:]:
            if z == prev + 1:
                prev = z
                continue
            runs.append((z0, prev - z0 + 1))
            z0 = prev = z
        runs.append((z0, prev - z0 + 1))
        if len(runs) >= 2:
            L = runs[0][1]
            s = runs[1][0] - runs[0][0]
            if (
                s > 0
                and all(r[1] == L for r in runs)
                and all(runs[i + 1][0] - runs[i][0] == s for i in range(len(runs) - 1))
            ):
                flat.append(dict(o=o, c=c, z0=runs[0][0], L=L, R=len(runs),
                                 C=1, dstep=0, rs_src=KADJ * s,
                                 rs_dst=(NZ + 1) * s, rs_rc=s))
                continue
        for z0, L in runs:
            flat.append(dict(o=o, c=c, z0=z0, L=L, R=1, C=1, dstep=0,
                             rs_src=0, rs_dst=0, rs_rc=0))

    flat.sort(key=lambda p: (p["L"], p["C"], p["dstep"], p["z0"]))
    ops = list(merged_ops)
    used = [False] * len(flat)
    for i in range(len(flat)):
        if used[i]:
            continue
        p = flat[i]
        if p["R"] != 1:
            ops.append(p)
            used[i] = True
            continue
        group = [p]
        for jx in range(i + 1, len(flat)):
            q = flat[jx]
            if not used[jx] and q["R"] == 1 and (
                (q["L"], q["C"], q["dstep"]) == (p["L"], p["C"], p["dstep"])
            ):
                group.append(q)
        if len(group) >= 2:
            group.sort(key=lambda g: (NZ + 1) * g["z0"] + g["o"])
            g0, g1 = group[0], group[1]
            ds = (NZ + 1) * (g1["z0"] - g0["z0"]) + (g1["o"] - g0["o"])
            ss = KADJ * (g1["z0"] - g0["z0"]) + (g1["c"] - g0["c"])
            rs = g1["z0"] - g0["z0"]
            okn = 1
            for t in range(1, len(group)):
                ga, gb = group[t - 1], group[t]
                if (
                    (NZ + 1) * (gb["z0"] - ga["z0"]) + (gb["o"] - ga["o"]) == ds
                    and KADJ * (gb["z0"] - ga["z0"]) + (gb["c"] - ga["c"]) == ss
                    and gb["z0"] - ga["z0"] == rs
                ):
                    okn = t + 1
                else:
                    break
            if okn >= 2:
                for g in group[:okn]:
                    used[flat.index(g)] = True
                ops.append(
                    dict(o=g0["o"], c=g0["c"], z0=g0["z0"], L=g0["L"],
                         R=okn, C=g0["C"], dstep=g0["dstep"], rs_src=ss,
                         rs_dst=ds, rs_rc=rs)
                )
                continue
        used[i] = True
        ops.append(p)
    return assign, ops


def _build_wa(W, b, assign):
    W = np.asarray(W, np.float32)
    b = np.asarray(b, np.float32)
    # padded to SLOTSP columns; dead columns keep bias -1e9 -> exp == 0
    Wa = np.zeros((DA, SLOTSP), np.float32)
    Wa[D, :] = NEG                     # unassigned slots: exp -> exactly 0
    for n in range(NZ):
        for c, k in assign[n].items():
            col = KADJ * n + c
            Wa[:D, col] = W[n, :, k]
            Wa[D, col] = b[n, k]
    return Wa


def _build_program_fast(ops):
    from concourse import bacc, mybir
    from concourse.ap import AP
    import concourse.tile as tile

    f32 = mybir.dt.float32
    bf16 = mybir.dt.bfloat16
    u8 = mybir.dt.uint8
    AF = mybir.ActivationFunctionType
    OP = mybir.AluOpType
    nc = bacc.Bacc("TRN2", target_bir_lowering=False, debug=False)

    # matmul inputs in bf16: fp32 PE streams at half rate, and the input
    # DMA loads halve; logits still accumulate in f32 PSUM
    xTa_d = nc.declare_dram_parameter("xTa", [DA, BLOC], bf16, isOutput=False)
    Wa_d = nc.declare_dram_parameter("Wa", [DA, SLOTSP], bf16, isOutput=False)
    out_d = nc.declare_dram_parameter("out", [BLOC, OUTW], u8, isOutput=True)

    with tile.TileContext(nc) as tc:
        with (
            tc.tile_pool(name="const", bufs=1) as cpool,
            tc.tile_pool(name="ework", bufs=1) as epool,
            tc.tile_pool(name="dwork", bufs=1) as dpool,
            tc.tile_pool(name="ps_lg", bufs=1, space="PSUM") as ps_lg,
        ):
            # per-partition scalar bias ln(254) for the scaled exp, plus a
            # warmup activation so the one-time ACT exp-table load (~1.3us)
            # overlaps the input DMAs instead of gating the first real exp
            lnq = cpool.tile([P, 1], f32, tag="lnq")
            nc.gpsimd.memset(lnq[:, :], float(np.log(QSCALE)))
            warm = cpool.tile([P, 1], f32, tag="warm")
            nc.scalar.activation(warm[:, :], lnq[:, :], AF.Exp)

            xTa_sb = cpool.tile([DA, BLOC], bf16, tag="xTa")
            # chunked input load, all up front (interleaving input reads into
            # the output write stream measurably slows the DMA engines):
            # Wa + a small first chunk so sub-block 0's matmul starts early
            xbounds = [0, 128, 1024, BLOC]
            xchunks = list(zip(xbounds, xbounds[1:]))

            def load_chunk(j, eng):
                lo, hi = xchunks[j]
                eng.dma_start(out=xTa_sb[:, lo:hi], in_=xTa_d[:, lo:hi])

            # Wa + chunk0 on the scalar HWDGE ring (it is free ~1.5us before
            # the sync ring finishes its TileContext scope-entry), the big
            # chunks on the sync ring: MM0 starts as early as possible
            Wa_sb = cpool.tile([DA, SLOTSP], bf16, tag="Wa")
            nc.scalar.dma_start(out=Wa_sb[:], in_=Wa_d[:])
            load_chunk(0, nc.scalar)
            for j in range(1, len(xchunks)):
                load_chunk(j, nc.sync)

            # persistent DOUBLE-WIDE u8 output tiles (one per block PAIR),
            # zeroed once, up front, split across DVE and GPSIMD: serial
            # GPSIMD-only memsets (2.8us each) gated the first scatters and
            # delayed output-DMA saturation by ~5us; DVE is idle until the
            # first reduce (~13us) so it can zero two tiles for free
            # ALL memsets on GPSIMD, in consumption order: any DVE memset
            # sits in-order ahead of the first reduce and delays the whole
            # output stream; GPSIMD is idle for the entire ramp and zeroes
            # tile j well before pair j's scatter needs it.
            osb = []
            for j in range(NOSB):
                ot = cpool.tile([P, 2 * OSBW], u8, tag=f"osb{j}")
                osb.append(ot)
            for j in range(NOSB):
                nc.gpsimd.memset(osb[j][:, :].bitcast(f32), 0.0)

            # Process block PAIRS: per-instruction fixed costs (~0.4-0.7us)
            # and cross-engine semaphore edges dominate at u8 rates, so one
            # reduce/reciprocal/scatter-op covers 2 sub-blocks via an extra
            # AP dim, and one 1.68 MB DMA writes 256 rows. PSUM tile = 2
            # banks (each half's matmul writes its own bank); 4 tags fill
            # the 8 PSUM banks for 4 pairs in flight.
            NPAIR = NSUB // 2
            for pr in range(NPAIR):
                lg = ps_lg.tile([P, 2 * SLOTSP], f32, tag=f"lg{pr % 4}")
                e2 = epool.tile([P, 2 * SLOTSP], f32, tag=f"e{pr % 4}")
                e254 = epool.tile([P, 2 * SLOTSP], f32, tag=f"e254_{pr % 4}")
                for h in range(2):
                    i = 2 * pr + h
                    nc.tensor.matmul(
                        lg[:, h * SLOTSP:(h + 1) * SLOTSP],
                        xTa_sb[:, i * P:(i + 1) * P],
                        Wa_sb[:, :],
                        start=True,
                        stop=True,
                    )
                # ONE exp of each kind over the whole [P, 1024] pair tile
                # (dead slots have bias -1e9 -> exp 0): halving the ACT
                # instruction count halves the PE->ACT semaphore waits,
                # which were serializing the cross-pair pipeline. e for
                # the denominator reduce, e254 = exp(lg + ln 254) = 254*e
                # for the scatter numerator.
                nc.scalar.activation(e2[:, :], lg[:, :], AF.Exp)
                nc.scalar.activation(e254[:, :], lg[:, :], AF.Exp, bias=lnq[:, :])
                den = dpool.tile([P, 2 * NZ], f32, tag=f"den{pr % 4}")
                nc.vector.tensor_reduce(
                    AP(
                        tensor=den[:, :].tensor,
                        offset=0,
                        ap=[[2 * NZ, P], [NZ, 2], [1, NZ]],
                    ),
                    AP(
                        tensor=e2[:, :].tensor,
                        offset=0,
                        ap=[[2 * SLOTSP, P], [SLOTSP, 2], [KADJ, NZ],
                            [1, KADJ]],
                    ),
                    mybir.AxisListType.X,
                    OP.add,
                )
                rc = dpool.tile([P, 2 * NZ], f32, tag=f"rc{pr % 4}")
                # ~5x faster than reciprocal(); 18-bit accuracy is plenty
                # for the u8 output, and den is in [~0.05, ~500] so the
                # undefined edge cases (0/denorm/inf) cannot occur
                nc.vector.reciprocal_approx_fast(out=rc[:, :], in_=den[:, :])

                ot = osb[pr % NOSB]
                # all scatter ops on DVE: f32*f32 -> u8 converts with
                # round-to-nearest-even + saturation; GPSIMD (Pool) cannot
                # mix dtypes on integer tensor_tensor ops. The pair dim
                # rides the AP (dims with count 1 are dropped to stay
                # within 4; R>1 and C>1 together fall back to per-half).
                for op in ops:
                    o, c, z0, L, R, C, dstep = (
                        op["o"], op["c"], op["z0"], op["L"], op["R"],
                        op["C"], op["dstep"],
                    )

                    def emit(pair_dims):
                        sdims, ddims, rdims, soff, doff, roff = pair_dims
                        src = AP(
                            tensor=e254[:, :].tensor,
                            offset=soff + KADJ * z0 + c,
                            ap=[[2 * SLOTSP, P]] + sdims,
                        )
                        dst = AP(
                            tensor=ot[:, :].tensor,
                            offset=doff + PADL + (NZ + 1) * z0 + o,
                            ap=[[2 * OSBW, P]] + ddims,
                        )
                        rca = AP(
                            tensor=rc[:, :].tensor,
                            offset=roff + z0,
                            ap=[[2 * NZ, P]] + rdims,
                        )
                        nc.vector.tensor_tensor(
                            out=dst, in0=src, in1=rca, op=OP.mult
                        )

                    sdims = [[op["rs_src"], R], [KADJ, L], [1, C]]
                    ddims = [[op["rs_dst"], R], [NZ + 1, L], [dstep, C]]
                    rdims = [[op["rs_rc"], R], [1, L], [0, C]]
                    keep = [k for k in range(3) if (R, L, C)[k] > 1]
                    if len(keep) <= 2:
                        emit((
                            [[SLOTSP, 2]] + [sdims[k] for k in keep],
                            [[OSBW, 2]] + [ddims[k] for k in keep],
                            [[NZ, 2]] + [rdims[k] for k in keep],
                            0, 0, 0,
                        ))
                    else:
                        for h in range(2):
                            emit((
                                [sdims[k] for k in keep],
                                [ddims[k] for k in keep],
                                [rdims[k] for k in keep],
                                h * SLOTSP, h * OSBW, h * NZ,
                            ))
                odst = AP(
                    tensor=out_d[:, :].tensor,
                    offset=pr * 2 * P * OUTW,
                    ap=[[OUTW, P], [P * OUTW, 2], [1, OUTW]],
                )
                osrc = AP(
                    tensor=ot[:, :].tensor,
                    offset=PADL,
                    ap=[[2 * OSBW, P], [OSBW, 2], [1, OUTW]],
                )
                nc.sync.dma_start(out=odst, in_=osrc)
    nc.compile()
    return nc


# --------------------------------------------------------------------------
# Fallback path (general scatter-add): dense scatter-matmul, f32 output
# --------------------------------------------------------------------------

BF = 512
ZPG = 6
NGRP = 14
GRP_NZ = [6] * 13 + [3]
GRP_COL = [486 * g for g in range(14)]
PW_PAIR = [128, 128, 128, 64]
PADW = 448


def _slot_mm(n, k):
    g = n // ZPG
    zz = n % ZPG
    hc = g // 2
    p = hc // 2
    row_hi = 32 * (g % 2) + KADJ * zz + k
    row_pair = 64 * (hc % 2) + row_hi
    return p, row_pair, hc, row_hi


def _build_consts_mm(W, b, idx, mask):
    import ml_dtypes

    bf = ml_dtypes.bfloat16
    W = np.asarray(W, np.float32)
    b = np.asarray(b, np.float32)
    idx = np.asarray(idx)
    mask = np.asarray(mask, np.float32)

    Wa = np.zeros((DA, PADW), np.float32)
    E = np.zeros((NZ, PADW), bf)
    ob = [np.zeros((PW_PAIR[p], NZ), np.float32) for p in range(4)]
    S = np.zeros((P, NZ * NZ), bf)

    for n in range(NZ):
        for k in range(KADJ):
            p, rp, hc, rh = _slot_mm(n, k)
            col = 128 * p + rp
            if mask[n, k] > 0:
                Wa[:D, col] = W[n, :, k]
                Wa[D, col] = b[n, k]
            else:
                Wa[D, col] = NEG
            E[n, col] = 1.0
            ob[p][rp, n] = 1.0
            ocol = n * NZ + int(idx[n, k])
            S[rh, ocol] = 1.0
            S[64 + rh, ocol] = 1.0
    return Wa, E, ob, S


def _build_program_mm(bloc):
    from concourse import bacc, mybir
    import concourse.tile as tile

    f32 = mybir.dt.float32
    bf16 = mybir.dt.bfloat16
    AF = mybir.ActivationFunctionType
    OP = mybir.AluOpType
    nc = bacc.Bacc("TRN2", target_bir_lowering=False, debug=False)

    xTa_d = nc.declare_dram_parameter("xTa", [DA, bloc], f32, isOutput=False)
    Wa_d = nc.declare_dram_parameter("Wa", [DA, PADW], f32, isOutput=False)
    E_d = nc.declare_dram_parameter("E", [NZ, PADW], bf16, isOutput=False)
    ob_d = [
        nc.declare_dram_parameter(f"ob{p}", [PW_PAIR[p], NZ], f32, isOutput=False)
        for p in range(4)
    ]
    S_d = nc.declare_dram_parameter("S", [P, NZ * NZ], bf16, isOutput=False)
    out_d = nc.declare_dram_parameter("out", [bloc, NZ * NZ], f32, isOutput=True)

    n_blk = bloc // BF
    n_sub = BF // P

    with tile.TileContext(nc) as tc:
        with (
            tc.tile_pool(name="const", bufs=1) as cpool,
            tc.tile_pool(name="work", bufs=2) as wpool,
            tc.tile_pool(name="outp", bufs=4) as opool,
            tc.tile_pool(name="ps_log", bufs=2, space="PSUM") as ps_log,
            tc.tile_pool(name="ps_den", bufs=1, space="PSUM") as ps_den,
            tc.tile_pool(name="ps_rf", bufs=2, space="PSUM") as ps_rf,
            tc.tile_pool(name="ps_sc", bufs=3, space="PSUM") as ps_sc,
        ):
            Wa_sb = cpool.tile([DA, PADW], f32, tag="Wa")
            nc.sync.dma_start(out=Wa_sb[:], in_=Wa_d[:])
            E_sb = cpool.tile([NZ, PADW], bf16, tag="E")
            nc.sync.dma_start(out=E_sb[:], in_=E_d[:])
            S_sb = cpool.tile([P, NZ * NZ], bf16, tag="S")
            nc.sync.dma_start(out=S_sb[:], in_=S_d[:])
            ob_sb = []
            for p in range(4):
                t = cpool.tile([PW_PAIR[p], NZ], f32, tag=f"ob{p}")
                nc.sync.dma_start(out=t[:], in_=ob_d[p][:])
                ob_sb.append(t)
            xTa_sb = cpool.tile([DA, bloc], f32, tag="xTa")
            nc.sync.dma_start(out=xTa_sb[:], in_=xTa_d[:])

            def emit_scatter(bs, pcat):
                for i in range(n_sub):
                    osb = opool.tile([P, NZ * NZ], f32, tag="osb")
                    for g in range(NGRP):
                        ncols = GRP_NZ[g] * NZ
                        colg = GRP_COL[g]
                        sc = ps_sc.tile([P, BF], f32, tag="scps")
                        nc.tensor.matmul(
                            sc[:, :ncols],
                            pcat[g // 2][:, i * P:(i + 1) * P],
                            S_sb[:, colg:colg + ncols],
                            start=True,
                            stop=True,
                        )
                        dst = osb[:, colg:colg + ncols]
                        if g % 5 < 3:
                            nc.scalar.copy(dst, sc[:, :ncols])
                        else:
                            nc.vector.tensor_copy(dst, sc[:, :ncols])
                    nc.sync.dma_start(
                        out=out_d[bs + i * P: bs + (i + 1) * P, :], in_=osb[:]
                    )

            prev = None
            for blk in range(n_blk):
                bs = blk * BF
                exT = []
                for p in range(4):
                    pw = PW_PAIR[p]
                    lg = ps_log.tile([P, BF], f32, tag="lg")
                    nc.tensor.matmul(
                        lg[:pw, :],
                        Wa_sb[:, 128 * p:128 * p + pw],
                        xTa_sb[:, bs:bs + BF],
                        start=True,
                        stop=True,
                    )
                    ex = wpool.tile([P, BF], f32, tag=f"exp{p}")
                    nc.scalar.activation(ex[:pw, :], lg[:pw, :], AF.Exp)
                    exT.append(ex)
                den_ps = ps_den.tile([NZ, BF], f32, tag="den")
                for p in range(4):
                    nc.tensor.matmul(
                        den_ps[:, :], ob_sb[p][:], exT[p][:PW_PAIR[p], :],
                        start=(p == 0), stop=(p == 3),
                    )
                rc = wpool.tile([NZ, BF], f32, tag="recipC")
                nc.vector.reciprocal(rc[:], den_ps[:])
                rhi = wpool.tile([NZ, BF], bf16, tag="rhi")
                nc.scalar.copy(rhi[:], rc[:])
                rlo = wpool.tile([NZ, BF], bf16, tag="rlo")
                nc.vector.tensor_tensor(out=rlo[:], in0=rc[:], in1=rhi[:], op=OP.subtract)
                pcat = []
                for p in range(4):
                    pw = PW_PAIR[p]
                    rf = ps_rf.tile([P, BF], f32, tag="rf")
                    nc.tensor.matmul(
                        rf[:pw, :], E_sb[:, 128 * p:128 * p + pw], rhi[:],
                        start=True, stop=False,
                    )
                    nc.tensor.matmul(
                        rf[:pw, :], E_sb[:, 128 * p:128 * p + pw], rlo[:],
                        start=False, stop=True,
                    )
                    for h in range(2 if pw == 128 else 1):
                        sl = slice(64 * h, 64 * h + 64)
                        pt = wpool.tile([64, BF], f32, tag=f"pt{2 * p + h}")
                        nc.vector.tensor_tensor(
                            out=pt[:, :], in0=exT[p][sl, :], in1=rf[sl, :], op=OP.mult
                        )
                        pc = wpool.tile([P, BF], bf16, tag=f"pcat{2 * p + h}")
                        nc.scalar.copy(pc[:64, :], pt[:, :])
                        nc.vector.tensor_tensor(
                            out=pc[64:, :],
                            in0=pt[:, :],
                            in1=pc[:64, :],
                            op=OP.subtract,
                        )
                        pcat.append(pc)
                if prev is not None:
                    emit_scatter(*prev)
                prev = (bs, pcat)
            emit_scatter(*prev)
    nc.compile()
    return nc


# --------------------------------------------------------------------------
# Entry
# --------------------------------------------------------------------------

def _install_ntff_hook():
    """Shim antenv.axon_hooks (absent in this image) so trace=True can drive
    NRT profiling through libaxon_pjrt.so. Only used for self-profiling."""
    import types

    try:
        import antenv

        try:
            from antenv.axon_hooks import get_axon_ntff_profile_hook  # noqa: F401

            return True
        except ImportError:
            pass
        if "/root/.axon_site" not in sys.path:
            sys.path.insert(0, "/root/.axon_site")
        from trn_agent_boot.trn_boot import _ntff_profile_via_ctypes

        hook = _ntff_profile_via_ctypes("/opt/axon/libaxon_pjrt.so")
        mod = types.ModuleType("antenv.axon_hooks")
        state = {"hook": hook}
        mod.get_axon_ntff_profile_hook = lambda: state["hook"]
        mod.set_axon_ntff_profile_hook = lambda h: state.update(hook=h)
        sys.modules["antenv.axon_hooks"] = mod
        antenv.axon_hooks = mod
        return hook is not None
    except Exception as e:  # profiling is best-effort; never break the run
        print("ntff hook install failed:", e)
        return False


def _make_xta_maps(obs, consts, dtype=np.float32):
    in_maps = []
    for i in range(NCORES):
        shard = obs[i * BLOC:(i + 1) * BLOC, :D]
        xTa = np.concatenate(
            [np.ascontiguousarray(shard.T), np.ones((1, BLOC), np.float32)], axis=0
        )
        m = dict(consts)
        m["xTa"] = np.ascontiguousarray(xTa).astype(dtype)
        in_maps.append(m)
    return in_maps


def kernel(obs, W, b, idx, mask):
    from concourse.bass_utils import run_bass_kernel_spmd

    global LAST_RESULTS
    trace = bool(int(os.environ.get("KBT_TRACE", "0")))
    if trace:
        trace = _install_ntff_hook()
    obs = np.asarray(obs, np.float32)
    idx = np.asarray(idx)
    mask = np.asarray(mask, np.float32)

    plan = _plan_scatter(idx, mask)
    if plan is not None:
        assign, ops = plan
        Wa = _build_wa(W, b, assign)
        nc = _build_program_fast(ops)
        import ml_dtypes

        bf = ml_dtypes.bfloat16
        in_maps = _make_xta_maps(obs, {"Wa": Wa.astype(bf)}, dtype=bf)
        br = run_bass_kernel_spmd(nc, in_maps, list(range(NCORES)), trace=trace)
        LAST_RESULTS = br
        # dequantize u8 fixed-point (value = q / 254) via LUT
        lut = (np.arange(256, dtype=np.float32) * np.float32(1.0 / QSCALE))
        out = np.empty((BATCH, OUTW), np.float32)
        for i in range(NCORES):
            np.take(
                lut,
                np.asarray(br.results[i]["out"]),
                out=out[i * BLOC:(i + 1) * BLOC],
            )
        return out.reshape(BATCH, NZ, NZ)

    # general scatter-add fallback
    Wa, E, ob, S = _build_consts_mm(W, b, idx, mask)
    nc = _build_program_mm(BLOC)
    consts = {"Wa": Wa, "E": E, "S": S}
    for p in range(4):
        consts[f"ob{p}"] = ob[p]
    in_maps = _make_xta_maps(obs, consts)
    br = run_bass_kernel_spmd(nc, in_maps, list(range(NCORES)), trace=trace)
    LAST_RESULTS = br
    out = np.concatenate([br.results[i]["out"] for i in range(NCORES)], axis=0)
    return out.reshape(BATCH, NZ, NZ)



# revision 38
# speedup vs baseline: 1.0275x; 1.0275x over previous
"""Trainium2 Bass kernel for nn_CollectiveDecActorTaxi0Obs (gnn_message_passing).

Computes, for obs [32768, 48], per-zone dense heads W [81, 48, 5] (+bias b,
adjacency idx/mask [81, 5]):
    logits = einsum('bd,ndk->bnk', obs, W) + b ; masked softmax over k
    out[b, n, idx[n, k]] += probs[b, n, k]              -> [32768, 81, 81] f32

Strategy (pure data parallelism, 8 cores, batch-sharded 4096 rows each):
  The kernel is HBM-write-bound: the output is 860 MB dense but within the
  2e-2 tolerance, so the device writes it as u8 fixed-point (prob*254,
  max quant err 0.5/254 ~ 0.002 << 0.019 tolerance; DVE f32->u8 conversion
  rounds-to-nearest-even and saturates). 215 MB total, ~27 MB/core, ~75 us
  at the ~358 GB/s per-core HBM limit. The host dequantizes via LUT.

  Everything runs with batch on the PARTITION dim in 32 sub-blocks of 128
  rows per core:
    - logits: one [49,128]^T @ [49,405] f32 matmul per sub-block (weights
      Wa pack all 81 zones' 5 slot columns + a bias row; masked slots get
      bias -1e9 so exp underflows to exactly 0).
    - exp on the scalar engine (PSUM -> SBUF), per-zone denominator via a
      window-5 tensor_reduce on GPSIMD + scale by 1/254 there, reciprocal
      on DVE (so rc = 254/den).
    - The scatter out[b, n, idx[n,k]] is batch-invariant: only ~405 of the
      6561 output columns are ever nonzero. Output tiles [128, 6561] u8
      live persistently in SBUF, memset to zero ONCE (halves split across
      DVE/GPSIMD, interleaved with the first sub-blocks); each sub-block
      just rewrites the hot columns with strided e*rc ops (dst stride 82
      on the zone-diagonal, classes hull-extended to single runs by writing
      computed zeros over never-hot columns), all on DVE (GPSIMD cannot
      convert f32->u8: integer TT on Pool requires matching dtypes), then
      DMAs the dense tile. For the grid adjacency this is 5 flat strided
      ops per sub-block.

  The host plans slot classes generically from idx/mask; if a zone has
  duplicate destinations (scatter-add semantics), it falls back to a dense
  scatter-matmul path (probs @ 0/1 S matrix, f32 output).
"""

import os
import sys

sys.path.insert(0, "/opt/trn_rl_repo")

import numpy as np

NZ = 81          # zones
D = 48           # obs dim used
DA = D + 1       # + bias row
KADJ = 5         # adjacency slots per zone
NCORES = 8
BATCH = 32768
BLOC = BATCH // NCORES   # 4096 rows per core
P = 128
NSUB = BLOC // P         # 32 sub-blocks of 128 batch rows
SLOTS = NZ * KADJ        # 405 packed slot columns
SLOTSP = 512             # padded slot pitch: one PSUM bank of f32 per half
OUTW = NZ * NZ           # 6561 output columns
PADL = 4                 # osb left pad: lets merged ops write col -1..-4
PADR = 3                 # osb right pad (also rounds width to mult of 4)
OSBW = PADL + OUTW + PADR
NOSB = 6                 # persistent output staging buffers (block pairs)
NEG = np.float32(-1e9)
QSCALE = np.float32(254.0)  # u8 fixed-point scale for probs in [0, 1]

LAST_RESULTS = None


# --------------------------------------------------------------------------
# Fast path: class-slot planning + strided-scatter program
# --------------------------------------------------------------------------

def _plan_scatter(idx, mask):
    """Assign each valid (zone, k) a slot class c so that zones sharing a
    destination offset o = idx-n share c, then group (o, c) classes into
    strided ops. Returns (assign, ops) or None if any zone has duplicate
    destinations (needs scatter-ADD, handled by the fallback path).

    assign: {n: {c: k}}   ops: [{o, c, z0, L, R, s}] meaning zones
    z0 + i*s + j for i<R, j<L write probs[:, 5*(z)+c] to out col 82*z + o.
    """
    from collections import Counter

    byzone = {}
    for n in range(NZ):
        dests = set()
        for k in range(KADJ):
            if mask[n, k] > 0:
                d = int(idx[n, k])
                if d in dests:
                    return None
                dests.add(d)
                byzone.setdefault(n, []).append((k, d - n))

    # Slot assignment: the (up to 5) globally most common offsets get slot
    # index = their rank in ASCENDING offset order, so classes with adjacent
    # offsets sit in adjacent slots and can later chain into one op.
    cnt = Counter(o for lst in byzone.values() for (_, o) in lst)
    top = [o for o, _ in cnt.most_common(KADJ)]
    pref = {o: r for r, o in enumerate(sorted(top))}

    assign = {n: {} for n in range(NZ)}
    offs = {n: set(o for (_, o) in byzone.get(n, [])) for n in range(NZ)}
    classes = {}
    for n in range(NZ):
        used, rest = set(), []
        for k, o in byzone.get(n, []):
            c = pref.get(o, KADJ)
            if c < KADJ and c not in used:
                used.add(c)
                assign[n][c] = k
                classes.setdefault((o, c), []).append(n)
            else:
                rest.append((k, o))
        free = [c for c in range(KADJ) if c not in used]
        for (k, o), c in zip(rest, free):
            assign[n][c] = k
            classes.setdefault((o, c), []).append(n)

    def cell_ok(o, c, z, zone_set):
        """May an op write cell (z, 82z+o+PADL) from slot (z, 5z+c)? Yes if
        z is a class member; else we'd write a computed zero (slot c must be
        unassigned there so Wa bias -1e9 -> exp 0), the zone must have some
        valid slot (else rc is inf -> 0*inf = NaN), and an in-row cell must
        not shadow another slot's destination column. Out-of-row cells land
        in the osb pad bytes (never DMA'd) and are always harmless."""
        col = (NZ + 1) * z + o
        if col < -PADL or col > OUTW - 1 + PADR:
            return False
        if z in zone_set:
            return True
        if not byzone.get(z):
            return False
        if c in assign[z]:
            return False
        if col < 0 or col > OUTW - 1:
            return True
        return o not in offs[z]

    # Per-instruction fixed cost (~0.4-0.6 us) dominates these ops, so fold
    # as many classes as possible into single instructions:
    #  Pass 1  C-chain merge: classes (o0+j, c0+j) share one op whose inner
    #          dim steps both src slot and dst column by 1 (contiguous).
    #  Pass 2  leftover classes -> single-run hull or uniform runs.
    #  Pass 3  R-merge ops with equal (L, C) via independent per-AP strides.
    merged_ops = []
    consumed = set()
    items = sorted(classes.items())
    cls = {oc: sorted(zs) for oc, zs in items}
    keys = set(cls)
    for (o0, c0) in sorted(keys):
        if (o0, c0) in consumed:
            continue
        chain = [(o0, c0)]
        while (chain[-1][0] + 1, chain[-1][1] + 1) in keys and (
            chain[-1][0] + 1, chain[-1][1] + 1
        ) not in consumed:
            chain.append((chain[-1][0] + 1, chain[-1][1] + 1))
        while len(chain) >= 2:
            A = min(cls[oc][0] for oc in chain)
            B = max(cls[oc][-1] for oc in chain)
            if all(
                cell_ok(o, c, z, set(cls[(o, c)]))
                for (o, c) in chain
                for z in range(A, B + 1)
            ):
                merged_ops.append(
                    dict(o=o0, c=c0, z0=A, L=B - A + 1, R=1, C=len(chain),
                         dstep=1, rs_src=0, rs_dst=0, rs_rc=0)
                )
                consumed.update(chain)
                break
            chain.pop()

    flat = []
    for (o, c), zones in items:
        if (o, c) in consumed:
            continue
        zone_set = set(zones)
        a, b = zones[0], zones[-1]
        if all(cell_ok(o, c, z, zone_set) for z in range(a, b + 1)):
            flat.append(dict(o=o, c=c, z0=a, L=b - a + 1, R=1, C=1,
                             dstep=0, rs_src=0, rs_dst=0, rs_rc=0))
            continue
        runs, z0, prev = [], zones[0], zones[0]
        for z in zones[1:]:
            if z == prev + 1:
                prev = z
                continue
            runs.append((z0, prev - z0 + 1))
            z0 = prev = z
        runs.append((z0, prev - z0 + 1))
        if len(runs) >= 2:
            L = runs[0][1]
            s = runs[1][0] - runs[0][0]
            if (
                s > 0
                and all(r[1] == L for r in runs)
                and all(runs[i + 1][0] - runs[i][0] == s for i in range(len(runs) - 1))
            ):
                flat.append(dict(o=o, c=c, z0=runs[0][0], L=L, R=len(runs),
                                 C=1, dstep=0, rs_src=KADJ * s,
                                 rs_dst=(NZ + 1) * s, rs_rc=s))
                continue
        for z0, L in runs:
            flat.append(dict(o=o, c=c, z0=z0, L=L, R=1, C=1, dstep=0,
                             rs_src=0, rs_dst=0, rs_rc=0))

    flat.sort(key=lambda p: (p["L"], p["C"], p["dstep"], p["z0"]))
    ops = list(merged_ops)
    used = [False] * len(flat)
    for i in range(len(flat)):
        if used[i]:
            continue
        p = flat[i]
        if p["R"] != 1:
            ops.append(p)
            used[i] = True
            continue
        group = [p]
        for jx in range(i + 1, len(flat)):
            q = flat[jx]
            if not used[jx] and q["R"] == 1 and (
                (q["L"], q["C"], q["dstep"]) == (p["L"], p["C"], p["dstep"])
            ):
                group.append(q)
        if len(group) >= 2:
            group.sort(key=lambda g: (NZ + 1) * g["z0"] + g["o"])
            g0, g1 = group[0], group[1]
            ds = (NZ + 1) * (g1["z0"] - g0["z0"]) + (g1["o"] - g0["o"])
            ss = KADJ * (g1["z0"] - g0["z0"]) + (g1["c"] - g0["c"])
            rs = g1["z0"] - g0["z0"]
            okn = 1
            for t in range(1, len(group)):
                ga, gb = group[t - 1], group[t]
                if (
                    (NZ + 1) * (gb["z0"] - ga["z0"]) + (gb["o"] - ga["o"]) == ds
                    and KADJ * (gb["z0"] - ga["z0"]) + (gb["c"] - ga["c"]) == ss
                    and gb["z0"] - ga["z0"] == rs
                ):
                    okn = t + 1
                else:
                    break
            if okn >= 2:
                for g in group[:okn]:
                    used[flat.index(g)] = True
                ops.append(
                    dict(o=g0["o"], c=g0["c"], z0=g0["z0"], L=g0["L"],
                         R=okn, C=g0["C"], dstep=g0["dstep"], rs_src=ss,
                         rs_dst=ds, rs_rc=rs)
                )
                continue
        used[i] = True
        ops.append(p)
    return assign, ops


def _build_wa(W, b, assign):
    W = np.asarray(W, np.float32)
    b = np.asarray(b, np.float32)
    # padded to SLOTSP columns; dead columns keep bias -1e9 -> exp == 0
    Wa = np.zeros((DA, SLOTSP), np.float32)
    Wa[D, :] = NEG                     # unassigned slots: exp -> exactly 0
    for n in range(NZ):
        for c, k in assign[n].items():
            col = KADJ * n + c
            Wa[:D, col] = W[n, :, k]
            Wa[D, col] = b[n, k]
    return Wa


def _build_program_fast(ops):
    from concourse import bacc, mybir
    from concourse.ap import AP
    import concourse.tile as tile

    f32 = mybir.dt.float32
    bf16 = mybir.dt.bfloat16
    u8 = mybir.dt.uint8
    AF = mybir.ActivationFunctionType
    OP = mybir.AluOpType
    nc = bacc.Bacc("TRN2", target_bir_lowering=False, debug=False)

    # matmul inputs in bf16: fp32 PE streams at half rate, and the input
    # DMA loads halve; logits still accumulate in f32 PSUM
    xTa_d = nc.declare_dram_parameter("xTa", [DA, BLOC], bf16, isOutput=False)
    Wa_d = nc.declare_dram_parameter("Wa", [DA, SLOTSP], bf16, isOutput=False)
    out_d = nc.declare_dram_parameter("out", [BLOC, OUTW], u8, isOutput=True)

    with tile.TileContext(nc) as tc:
        with (
            tc.tile_pool(name="const", bufs=1) as cpool,
            tc.tile_pool(name="ework", bufs=1) as epool,
            tc.tile_pool(name="dwork", bufs=1) as dpool,
            tc.tile_pool(name="ps_lg", bufs=1, space="PSUM") as ps_lg,
        ):
            # per-partition scalar bias ln(254) for the scaled exp, plus a
            # warmup activation so the one-time ACT exp-table load (~1.3us)
            # overlaps the input DMAs instead of gating the first real exp
            lnq = cpool.tile([P, 1], f32, tag="lnq")
            nc.gpsimd.memset(lnq[:, :], float(np.log(QSCALE)))
            warm = cpool.tile([P, 1], f32, tag="warm")
            nc.scalar.activation(warm[:, :], lnq[:, :], AF.Exp)

            xTa_sb = cpool.tile([DA, BLOC], bf16, tag="xTa")
            # chunked input load, all up front (interleaving input reads into
            # the output write stream measurably slows the DMA engines):
            # Wa + a small first chunk so sub-block 0's matmul starts early
            xbounds = [0, 128, 1024, BLOC]
            xchunks = list(zip(xbounds, xbounds[1:]))

            def load_chunk(j, eng):
                lo, hi = xchunks[j]
                eng.dma_start(out=xTa_sb[:, lo:hi], in_=xTa_d[:, lo:hi])

            # Wa on the scalar HWDGE ring, xTa chunks on the sync ring: the
            # two first loads run in parallel so MM0 starts as early as
            # possible (chunk0 queued behind Wa on one ring lands ~2us later)
            Wa_sb = cpool.tile([DA, SLOTSP], bf16, tag="Wa")
            nc.scalar.dma_start(out=Wa_sb[:], in_=Wa_d[:])
            for j in range(len(xchunks)):
                load_chunk(j, nc.sync)

            # persistent DOUBLE-WIDE u8 output tiles (one per block PAIR),
            # zeroed once, up front, split across DVE and GPSIMD: serial
            # GPSIMD-only memsets (2.8us each) gated the first scatters and
            # delayed output-DMA saturation by ~5us; DVE is idle until the
            # first reduce (~13us) so it can zero two tiles for free
            # ALL memsets on GPSIMD, in consumption order: any DVE memset
            # sits in-order ahead of the first reduce and delays the whole
            # output stream; GPSIMD is idle for the entire ramp and zeroes
            # tile j well before pair j's scatter needs it.
            osb = []
            for j in range(NOSB):
                ot = cpool.tile([P, 2 * OSBW], u8, tag=f"osb{j}")
                osb.append(ot)
            for j in range(NOSB):
                nc.gpsimd.memset(osb[j][:, :].bitcast(f32), 0.0)

            # Process block PAIRS: per-instruction fixed costs (~0.4-0.7us)
            # and cross-engine semaphore edges dominate at u8 rates, so one
            # reduce/reciprocal/scatter-op covers 2 sub-blocks via an extra
            # AP dim, and one 1.68 MB DMA writes 256 rows. PSUM tile = 2
            # banks (each half's matmul writes its own bank); 4 tags fill
            # the 8 PSUM banks for 4 pairs in flight.
            NPAIR = NSUB // 2
            for pr in range(NPAIR):
                lg = ps_lg.tile([P, 2 * SLOTSP], f32, tag=f"lg{pr % 4}")
                e2 = epool.tile([P, 2 * SLOTSP], f32, tag=f"e{pr % 4}")
                e254 = epool.tile([P, 2 * SLOTSP], f32, tag=f"e254_{pr % 4}")
                for h in range(2):
                    i = 2 * pr + h
                    nc.tensor.matmul(
                        lg[:, h * SLOTSP:(h + 1) * SLOTSP],
                        xTa_sb[:, i * P:(i + 1) * P],
                        Wa_sb[:, :],
                        start=True,
                        stop=True,
                    )
                # ONE exp of each kind over the whole [P, 1024] pair tile
                # (dead slots have bias -1e9 -> exp 0): halving the ACT
                # instruction count halves the PE->ACT semaphore waits,
                # which were serializing the cross-pair pipeline. e for
                # the denominator reduce, e254 = exp(lg + ln 254) = 254*e
                # for the scatter numerator.
                nc.scalar.activation(e2[:, :], lg[:, :], AF.Exp)
                nc.scalar.activation(e254[:, :], lg[:, :], AF.Exp, bias=lnq[:, :])
                den = dpool.tile([P, 2 * NZ], f32, tag=f"den{pr % 4}")
                nc.vector.tensor_reduce(
                    AP(
                        tensor=den[:, :].tensor,
                        offset=0,
                        ap=[[2 * NZ, P], [NZ, 2], [1, NZ]],
                    ),
                    AP(
                        tensor=e2[:, :].tensor,
                        offset=0,
                        ap=[[2 * SLOTSP, P], [SLOTSP, 2], [KADJ, NZ],
                            [1, KADJ]],
                    ),
                    mybir.AxisListType.X,
                    OP.add,
                )
                rc = dpool.tile([P, 2 * NZ], f32, tag=f"rc{pr % 4}")
                # ~5x faster than reciprocal(); 18-bit accuracy is plenty
                # for the u8 output, and den is in [~0.05, ~500] so the
                # undefined edge cases (0/denorm/inf) cannot occur
                nc.vector.reciprocal_approx_fast(out=rc[:, :], in_=den[:, :])

                ot = osb[pr % NOSB]
                # all scatter ops on DVE: f32*f32 -> u8 converts with
                # round-to-nearest-even + saturation; GPSIMD (Pool) cannot
                # mix dtypes on integer tensor_tensor ops. The pair dim
                # rides the AP (dims with count 1 are dropped to stay
                # within 4; R>1 and C>1 together fall back to per-half).
                for op in ops:
                    o, c, z0, L, R, C, dstep = (
                        op["o"], op["c"], op["z0"], op["L"], op["R"],
                        op["C"], op["dstep"],
                    )

                    def emit(pair_dims):
                        sdims, ddims, rdims, soff, doff, roff = pair_dims
                        src = AP(
                            tensor=e254[:, :].tensor,
                            offset=soff + KADJ * z0 + c,
                            ap=[[2 * SLOTSP, P]] + sdims,
                        )
                        dst = AP(
                            tensor=ot[:, :].tensor,
                            offset=doff + PADL + (NZ + 1) * z0 + o,
                            ap=[[2 * OSBW, P]] + ddims,
                        )
                        rca = AP(
                            tensor=rc[:, :].tensor,
                            offset=roff + z0,
                            ap=[[2 * NZ, P]] + rdims,
                        )
                        nc.vector.tensor_tensor(
                            out=dst, in0=src, in1=rca, op=OP.mult
                        )

                    sdims = [[op["rs_src"], R], [KADJ, L], [1, C]]
                    ddims = [[op["rs_dst"], R], [NZ + 1, L], [dstep, C]]
                    rdims = [[op["rs_rc"], R], [1, L], [0, C]]
                    keep = [k for k in range(3) if (R, L, C)[k] > 1]
                    if len(keep) <= 2:
                        emit((
                            [[SLOTSP, 2]] + [sdims[k] for k in keep],
                            [[OSBW, 2]] + [ddims[k] for k in keep],
                            [[NZ, 2]] + [rdims[k] for k in keep],
                            0, 0, 0,
                        ))
                    else:
                        for h in range(2):
                            emit((
                                [sdims[k] for k in keep],
                                [ddims[k] for k in keep],
                                [rdims[k] for k in keep],
                                h * SLOTSP, h * OSBW, h * NZ,
                            ))
                odst = AP(
                    tensor=out_d[:, :].tensor,
                    offset=pr * 2 * P * OUTW,
                    ap=[[OUTW, P], [P * OUTW, 2], [1, OUTW]],
                )
                osrc = AP(
                    tensor=ot[:, :].tensor,
                    offset=PADL,
                    ap=[[2 * OSBW, P], [OSBW, 2], [1, OUTW]],
                )
                nc.sync.dma_start(out=odst, in_=osrc)
    nc.compile()
    return nc


# --------------------------------------------------------------------------
# Fallback path (general scatter-add): dense scatter-matmul, f32 output
# --------------------------------------------------------------------------

BF = 512
ZPG = 6
NGRP = 14
GRP_NZ = [6] * 13 + [3]
GRP_COL = [486 * g for g in range(14)]
PW_PAIR = [128, 128, 128, 64]
PADW = 448


def _slot_mm(n, k):
    g = n // ZPG
    zz = n % ZPG
    hc = g // 2
    p = hc // 2
    row_hi = 32 * (g % 2) + KADJ * zz + k
    row_pair = 64 * (hc % 2) + row_hi
    return p, row_pair, hc, row_hi


def _build_consts_mm(W, b, idx, mask):
    import ml_dtypes

    bf = ml_dtypes.bfloat16
    W = np.asarray(W, np.float32)
    b = np.asarray(b, np.float32)
    idx = np.asarray(idx)
    mask = np.asarray(mask, np.float32)

    Wa = np.zeros((DA, PADW), np.float32)
    E = np.zeros((NZ, PADW), bf)
    ob = [np.zeros((PW_PAIR[p], NZ), np.float32) for p in range(4)]
    S = np.zeros((P, NZ * NZ), bf)

    for n in range(NZ):
        for k in range(KADJ):
            p, rp, hc, rh = _slot_mm(n, k)
            col = 128 * p + rp
            if mask[n, k] > 0:
                Wa[:D, col] = W[n, :, k]
                Wa[D, col] = b[n, k]
            else:
                Wa[D, col] = NEG
            E[n, col] = 1.0
            ob[p][rp, n] = 1.0
            ocol = n * NZ + int(idx[n, k])
            S[rh, ocol] = 1.0
            S[64 + rh, ocol] = 1.0
    return Wa, E, ob, S


def _build_program_mm(bloc):
    from concourse import bacc, mybir
    import concourse.tile as tile

    f32 = mybir.dt.float32
    bf16 = mybir.dt.bfloat16
    AF = mybir.ActivationFunctionType
    OP = mybir.AluOpType
    nc = bacc.Bacc("TRN2", target_bir_lowering=False, debug=False)

    xTa_d = nc.declare_dram_parameter("xTa", [DA, bloc], f32, isOutput=False)
    Wa_d = nc.declare_dram_parameter("Wa", [DA, PADW], f32, isOutput=False)
    E_d = nc.declare_dram_parameter("E", [NZ, PADW], bf16, isOutput=False)
    ob_d = [
        nc.declare_dram_parameter(f"ob{p}", [PW_PAIR[p], NZ], f32, isOutput=False)
        for p in range(4)
    ]
    S_d = nc.declare_dram_parameter("S", [P, NZ * NZ], bf16, isOutput=False)
    out_d = nc.declare_dram_parameter("out", [bloc, NZ * NZ], f32, isOutput=True)

    n_blk = bloc // BF
    n_sub = BF // P

    with tile.TileContext(nc) as tc:
        with (
            tc.tile_pool(name="const", bufs=1) as cpool,
            tc.tile_pool(name="work", bufs=2) as wpool,
            tc.tile_pool(name="outp", bufs=4) as opool,
            tc.tile_pool(name="ps_log", bufs=2, space="PSUM") as ps_log,
            tc.tile_pool(name="ps_den", bufs=1, space="PSUM") as ps_den,
            tc.tile_pool(name="ps_rf", bufs=2, space="PSUM") as ps_rf,
            tc.tile_pool(name="ps_sc", bufs=3, space="PSUM") as ps_sc,
        ):
            Wa_sb = cpool.tile([DA, PADW], f32, tag="Wa")
            nc.sync.dma_start(out=Wa_sb[:], in_=Wa_d[:])
            E_sb = cpool.tile([NZ, PADW], bf16, tag="E")
            nc.sync.dma_start(out=E_sb[:], in_=E_d[:])
            S_sb = cpool.tile([P, NZ * NZ], bf16, tag="S")
            nc.sync.dma_start(out=S_sb[:], in_=S_d[:])
            ob_sb = []
            for p in range(4):
                t = cpool.tile([PW_PAIR[p], NZ], f32, tag=f"ob{p}")
                nc.sync.dma_start(out=t[:], in_=ob_d[p][:])
                ob_sb.append(t)
            xTa_sb = cpool.tile([DA, bloc], f32, tag="xTa")
            nc.sync.dma_start(out=xTa_sb[:], in_=xTa_d[:])

            def emit_scatter(bs, pcat):
                for i in range(n_sub):
                    osb = opool.tile([P, NZ * NZ], f32, tag="osb")
                    for g in range(NGRP):
                        ncols = GRP_NZ[g] * NZ
                        colg = GRP_COL[g]
                        sc = ps_sc.tile([P, BF], f32, tag="scps")
                        nc.tensor.matmul(
                            sc[:, :ncols],
                            pcat[g // 2][:, i * P:(i + 1) * P],
                            S_sb[:, colg:colg + ncols],
                            start=True,
                            stop=True,
                        )
                        dst = osb[:, colg:colg + ncols]
                        if g % 5 < 3:
                            nc.scalar.copy(dst, sc[:, :ncols])
                        else:
                            nc.vector.tensor_copy(dst, sc[:, :ncols])
                    nc.sync.dma_start(
                        out=out_d[bs + i * P: bs + (i + 1) * P, :], in_=osb[:]
                    )

            prev = None
            for blk in range(n_blk):
                bs = blk * BF
                exT = []
                for p in range(4):
                    pw = PW_PAIR[p]
                    lg = ps_log.tile([P, BF], f32, tag="lg")
                    nc.tensor.matmul(
                        lg[:pw, :],
                        Wa_sb[:, 128 * p:128 * p + pw],
                        xTa_sb[:, bs:bs + BF],
                        start=True,
                        stop=True,
                    )
                    ex = wpool.tile([P, BF], f32, tag=f"exp{p}")
                    nc.scalar.activation(ex[:pw, :], lg[:pw, :], AF.Exp)
                    exT.append(ex)
                den_ps = ps_den.tile([NZ, BF], f32, tag="den")
                for p in range(4):
                    nc.tensor.matmul(
                        den_ps[:, :], ob_sb[p][:], exT[p][:PW_PAIR[p], :],
                        start=(p == 0), stop=(p == 3),
                    )
                rc = wpool.tile([NZ, BF], f32, tag="recipC")
                nc.vector.reciprocal(rc[:], den_ps[:])
                rhi = wpool.tile([NZ, BF], bf16, tag="rhi")
                nc.scalar.copy(rhi[:], rc[:])
                rlo = wpool.tile([NZ, BF], bf16, tag="rlo")
                nc.vector.tensor_tensor(out=rlo[:], in0=rc[:], in1=rhi[:], op=OP.subtract)
                pcat = []
                for p in range(4):
                    pw = PW_PAIR[p]
                    rf = ps_rf.tile([P, BF], f32, tag="rf")
                    nc.tensor.matmul(
                        rf[:pw, :], E_sb[:, 128 * p:128 * p + pw], rhi[:],
                        start=True, stop=False,
                    )
                    nc.tensor.matmul(
                        rf[:pw, :], E_sb[:, 128 * p:128 * p + pw], rlo[:],
                        start=False, stop=True,
                    )
                    for h in range(2 if pw == 128 else 1):
                        sl = slice(64 * h, 64 * h + 64)
                        pt = wpool.tile([64, BF], f32, tag=f"pt{2 * p + h}")
                        nc.vector.tensor_tensor(
                            out=pt[:, :], in0=exT[p][sl, :], in1=rf[sl, :], op=OP.mult
                        )
                        pc = wpool.tile([P, BF], bf16, tag=f"pcat{2 * p + h}")
                        nc.scalar.copy(pc[:64, :], pt[:, :])
                        nc.vector.tensor_tensor(
                            out=pc[64:, :],
                            in0=pt[:, :],
                            in1=pc[:64, :],
                            op=OP.subtract,
                        )
                        pcat.append(pc)
                if prev is not None:
                    emit_scatter(*prev)
                prev = (bs, pcat)
            emit_scatter(*prev)
    nc.compile()
    return nc


# --------------------------------------------------------------------------
# Entry
# --------------------------------------------------------------------------

def _install_ntff_hook():
    """Shim antenv.axon_hooks (absent in this image) so trace=True can drive
    NRT profiling through libaxon_pjrt.so. Only used for self-profiling."""
    import types

    try:
        import antenv

        try:
            from antenv.axon_hooks import get_axon_ntff_profile_hook  # noqa: F401

            return True
        except ImportError:
            pass
        if "/root/.axon_site" not in sys.path:
            sys.path.insert(0, "/root/.axon_site")
        from trn_agent_boot.trn_boot import _ntff_profile_via_ctypes

        hook = _ntff_profile_via_ctypes("/opt/axon/libaxon_pjrt.so")
        mod = types.ModuleType("antenv.axon_hooks")
        state = {"hook": hook}
        mod.get_axon_ntff_profile_hook = lambda: state["hook"]
        mod.set_axon_ntff_profile_hook = lambda h: state.update(hook=h)
        sys.modules["antenv.axon_hooks"] = mod
        antenv.axon_hooks = mod
        return hook is not None
    except Exception as e:  # profiling is best-effort; never break the run
        print("ntff hook install failed:", e)
        return False


def _make_xta_maps(obs, consts, dtype=np.float32):
    in_maps = []
    for i in range(NCORES):
        shard = obs[i * BLOC:(i + 1) * BLOC, :D]
        xTa = np.concatenate(
            [np.ascontiguousarray(shard.T), np.ones((1, BLOC), np.float32)], axis=0
        )
        m = dict(consts)
        m["xTa"] = np.ascontiguousarray(xTa).astype(dtype)
        in_maps.append(m)
    return in_maps


def kernel(obs, W, b, idx, mask):
    from concourse.bass_utils import run_bass_kernel_spmd

    global LAST_RESULTS
    trace = bool(int(os.environ.get("KBT_TRACE", "0")))
    if trace:
        trace = _install_ntff_hook()
    obs = np.asarray(obs, np.float32)
    idx = np.asarray(idx)
    mask = np.asarray(mask, np.float32)

    plan = _plan_scatter(idx, mask)
    if plan is not None:
        assign, ops = plan
        Wa = _build_wa(W, b, assign)
        nc = _build_program_fast(ops)
        import ml_dtypes

        bf = ml_dtypes.bfloat16
        in_maps = _make_xta_maps(obs, {"Wa": Wa.astype(bf)}, dtype=bf)
        br = run_bass_kernel_spmd(nc, in_maps, list(range(NCORES)), trace=trace)
        LAST_RESULTS = br
        # dequantize u8 fixed-point (value = q / 254) via LUT
        lut = (np.arange(256, dtype=np.float32) * np.float32(1.0 / QSCALE))
        out = np.empty((BATCH, OUTW), np.float32)
        for i in range(NCORES):
            np.take(
                lut,
                np.asarray(br.results[i]["out"]),
                out=out[i * BLOC:(i + 1) * BLOC],
            )
        return out.reshape(BATCH, NZ, NZ)

    # general scatter-add fallback
    Wa, E, ob, S = _build_consts_mm(W, b, idx, mask)
    nc = _build_program_mm(BLOC)
    consts = {"Wa": Wa, "E": E, "S": S}
    for p in range(4):
        consts[f"ob{p}"] = ob[p]
    in_maps = _make_xta_maps(obs, consts)
    br = run_bass_kernel_spmd(nc, in_maps, list(range(NCORES)), trace=trace)
    LAST_RESULTS = br
    out = np.concatenate([br.results[i]["out"] for i in range(NCORES)], axis=0)
    return out.reshape(BATCH, NZ, NZ)

